# revision 15
# baseline (speedup 1.0000x reference)
"""Trainium2 Bass kernel for nn_Attention_26628797235884.

12-head attention block (qkv proj + per-head RMS norm + 2D RoPE + softmax
attention + output proj), batch 8 x seq 1024 x dim 768, data-parallel over
batch across 8 NeuronCores (batch b -> core b, weights replicated).

v2: software-pipelined over the 6 head-pairs so the TensorEngine work of
pair p+1 (QKV matmuls, rms-norm group sums, rope) fills the PE idle time
while the ScalarEngine runs the softmax exps of pair p (ACT is the serial
bottleneck of attention at ~19us/pair). Other key points:
  - scores computed transposed [sk, sq] in two K=64 row-tiled matmuls that
    run concurrently in the PE array (head A rows 0:64, head B rows 64:128).
  - PV col-tiled: head A -> psum partitions 0:64, head B -> 64:128,
    concurrently (M=64 each); softmax denominators from four M=1 matmuls
    (ones lhsT) col-tiled at psum partitions 0/32/64/96 of one bank.
  - rms rsqrt via exp(-0.5*ln(x)) so the ACT engine never leaves the
    ln+exp activation-table set (sqrt would force a table reload per pair).
  - all PSUM traffic fits 8 banks: 2-slot [128,1024] ring (scores, qkv
    groups, rope-norm fields, denom broadcast) + pv(2) + den(1) + sq(1).
All matmuls bf16 (inputs pre-cast on host), fp32 accumulation.
"""

import sys

import numpy as np
import ml_dtypes

try:
    import concourse.bass as bass  # noqa: F401
except ImportError:  # pragma: no cover
    sys.path.insert(0, "/opt/trn_rl_repo")

import concourse.tile as tile
from concourse import bacc, mybir
from concourse.bass_utils import run_bass_kernel_spmd

BF16 = mybir.dt.bfloat16
F32 = mybir.dt.float32
AF = mybir.ActivationFunctionType
NP_BF16 = ml_dtypes.bfloat16

B, S, C, H, D = 8, 1024, 768, 12, 64
KT = C // 128           # 6 contraction tiles over the model dim
ST = S // 128           # 8 seq tiles
NPAIR = 6               # head pairs
NCORES = 8
EPS = 1e-6
PAIRSWAP32 = [i ^ 1 for i in range(32)]

_CACHE = {}
DEBUG = False


# --------------------------------------------------------------------------
# host-side constant prep
# --------------------------------------------------------------------------

def _rope_tables():
    ROPE_DIM, PT_SEQ, FT_SEQ, THETA = 32, 16, 32, 10000.0
    freqs = 1.0 / (THETA ** (np.arange(0, ROPE_DIM, 2, dtype=np.float32)[: ROPE_DIM // 2] / ROPE_DIM))
    t = np.arange(FT_SEQ, dtype=np.float32) / FT_SEQ * PT_SEQ
    f = np.einsum("i,j->ij", t, freqs)
    f = np.repeat(f, 2, axis=-1)
    fh = np.broadcast_to(f[:, None, :], (FT_SEQ, FT_SEQ, ROPE_DIM))
    fw = np.broadcast_to(f[None, :, :], (FT_SEQ, FT_SEQ, ROPE_DIM))
    f2 = np.concatenate([fh, fw], axis=-1).reshape(FT_SEQ * FT_SEQ, 2 * ROPE_DIM)
    return np.cos(f2).astype(np.float32), np.sin(f2).astype(np.float32)


def _prep_shared(qkv_w, qkv_b, q_norm_w, k_norm_w, proj_w, proj_b):
    f32 = np.float32
    cos, sin = _rope_tables()                 # [S, D]
    pair = np.arange(D) ^ 1
    sa = sin.copy()
    sa[:, 0::2] *= -1.0                       # sign-folded sin for rotate_half

    def mk(tab, w):                           # -> [128, S] bf16, 2 heads stacked
        t = (tab * w[None, :]).T.astype(f32)  # [64, S]
        return np.ascontiguousarray(np.vstack([t, t])).astype(NP_BF16)

    qw = np.asarray(q_norm_w, f32)
    kw = np.asarray(k_norm_w, f32)
    wqkT = np.asarray(qkv_w, f32)[: 2 * C].T  # [C, 2C]; cols: q-tiles then k-tiles
    wqk = np.concatenate(
        [np.concatenate([wqkT[:, 128 * p : 128 * p + 128],
                         wqkT[:, C + 128 * p : C + 128 * p + 128]], axis=1)[None]
         for p in range(NPAIR)], axis=0)      # [6, C, 256] pair-major
    shared = {
        "wqk": np.ascontiguousarray(wqk.reshape(NPAIR * C, 256)).astype(NP_BF16),
        "wvT": np.ascontiguousarray(np.asarray(qkv_w, f32)[2 * C :].T).astype(NP_BF16),
        "pwT": np.ascontiguousarray(np.asarray(proj_w, f32).T).astype(NP_BF16),
        "cosq": mk(cos, qw),
        "sinq": mk(sa, qw[pair]),
        "cosk": mk(cos, kw),
        "sink": mk(sa, kw[pair]),
    }
    b = np.asarray(qkv_b, f32)
    shared["bqk"] = np.ascontiguousarray(b[: 2 * C].reshape(2 * KT, 128).T)  # [128, 12]
    shared["vbias"] = np.ascontiguousarray(np.tile(b[2 * C :][None, :], (128, 1)))
    shared["pbias"] = np.ascontiguousarray(np.tile(np.asarray(proj_b, f32)[None, :], (128, 1)))
    shared["ones1"] = np.ones((128, 1), NP_BF16)
    # sqsel[:, 2*is_k + h2, :] is a [128, 8] indicator lhsT: partition d of an
    # m-tile's t^2 chunk (h2) accumulates into ssq row 4*is_k + 2*(d//64) + h2.
    dd = np.arange(128)
    sqsel = np.zeros((128, 4, 8), NP_BF16)
    for is_k in range(2):
        for h2 in range(2):
            sqsel[dd, 2 * is_k + h2, 4 * is_k + 2 * (dd // 64) + h2] = 1
    shared["sqsel"] = np.ascontiguousarray(sqsel.reshape(128, 32))
    # selq[:, 2*is_k + h2, :] is an [8, 128] lhsT broadcasting rinv row
    # 4*is_k + 2*(P//64) + h2 to field partition P (chunk h2).
    P = np.arange(128)
    selq = np.zeros((8, 4, 128), NP_BF16)
    for is_k in range(2):
        for h2 in range(2):
            selq[4 * is_k + 2 * (P // 64) + h2, 2 * is_k + h2, P] = 1
    shared["selq"] = np.ascontiguousarray(selq.reshape(8, 512))
    # selden[:, h2, :]: [97, 128] lhsT broadcasting inv-denominator psum row
    # (0 if P<64 else 64) + 32*h2 to field partition P (chunk h2).
    selden = np.zeros((97, 2, 128), NP_BF16)
    for h2 in range(2):
        selden[(P // 64) * 64 + 32 * h2, h2, P] = 1
    shared["selden"] = np.ascontiguousarray(selden.reshape(97, 256))
    return shared


# --------------------------------------------------------------------------
# device graph
# --------------------------------------------------------------------------

def _graph(tc, d, out_d, dbg=None):
    nc = tc.nc
    from contextlib import ExitStack

    with ExitStack() as big:
        main = big.enter_context(tc.tile_pool(name="main", bufs=1))
        work = big.enter_context(tc.tile_pool(name="work", bufs=1))

        # ---------------- persistent SBUF ----------------
        xT = main.tile([128, KT, S], BF16, tag="xT")
        wqk = [main.tile([128, KT, 256], BF16, tag=f"wqk{p}", name=f"wqk{p}") for p in range(NPAIR)]
        wvT = main.tile([128, KT, C], BF16, tag="wvT")
        pwT = main.tile([128, KT, C], BF16, tag="pwT")
        tabs = {}
        for nm in ("cosq", "sinq", "cosk", "sink"):
            tabs[nm] = main.tile([128, S], BF16, tag=nm, name=nm)
        bqk = main.tile([128, 2 * KT], F32, tag="bqk")
        vbias = main.tile([128, C], F32, tag="vbias")
        pbias = main.tile([128, C], F32, tag="pbias")
        ones1 = main.tile([128, 1], BF16, tag="ones1")
        sqsel = main.tile([128, 4, 8], BF16, tag="sqsel")
        selq = main.tile([8, 4, 128], BF16, tag="selq")
        selden = main.tile([97, 2, 128], BF16, tag="selden")
        v_sb = [main.tile([128, C], BF16, tag=f"v{j}", name=f"v{j}") for j in range(ST)]
        outT = [main.tile([128, S], BF16, tag=f"ot{p}", name=f"ot{p}") for p in range(NPAIR)]

        # ---------------- DMAs, priority order ----------------
        xT_r = d["xT"].rearrange("(k p) s -> p k s", p=128)
        for k in range(KT):
            nc.sync.dma_start(xT[:, k], xT_r[:, k])
        wqk_r = d["wqk"].rearrange("(pp kk p) c -> pp p kk c", pp=NPAIR, p=128)
        nc.sync.dma_start(wqk[0][:], wqk_r[0])
        nc.sync.dma_start(bqk[:], d["bqk"][:])
        for nm in tabs:
            nc.sync.dma_start(tabs[nm][:], d[nm][:])
        nc.sync.dma_start(sqsel[:], d["sqsel"].rearrange("p (b e) -> p b e", b=4))
        nc.sync.dma_start(selq[:], d["selq"].rearrange("p (b e) -> p b e", b=4))
        nc.sync.dma_start(ones1[:], d["ones1"][:])
        wvT_r = d["wvT"].rearrange("(k p) o -> p k o", p=128)
        for k in range(KT):
            nc.sync.dma_start(wvT[:, k], wvT_r[:, k])
        nc.sync.dma_start(vbias[:], d["vbias"][:])
        nc.sync.dma_start(wqk[1][:], wqk_r[1])
        nc.sync.dma_start(selden[:], d["selden"].rearrange("p (b e) -> p b e", b=2))
        for p in range(2, NPAIR):
            nc.sync.dma_start(wqk[p][:], wqk_r[p])
        pwT_r = d["pwT"].rearrange("(k p) o -> p k o", p=128)
        nc.sync.dma_start(pwT[:], pwT_r[:])
        nc.sync.dma_start(pbias[:], d["pbias"][:])

        st = {}  # per-pair pipeline state: t1/qk tiles etc.

        with ExitStack() as att:
            ring = att.enter_context(tc.tile_pool(name="ring", bufs=2, space="PSUM"))
            pvp = att.enter_context(tc.tile_pool(name="pvp", bufs=1, space="PSUM"))
            denp = att.enter_context(tc.tile_pool(name="denp", bufs=1, space="PSUM"))
            sqp = att.enter_context(tc.tile_pool(name="sqp", bufs=1, space="PSUM"))
            qkp = att.enter_context(tc.tile_pool(name="qkp", bufs=1))

            # ---------------- emission helpers ----------------
            def qkv_half(p, is_k, h2):
                # half-width QKV accumulation in its own ring slot so the PE
                # cost per attention j-slot stays under the exp period
                nm = f"qkv{p}{'k' if is_k else 'q'}{h2}"
                slot = ring.tile([128, S], F32, tag="ring", name=nm)
                cs = slice(512 * h2, 512 * h2 + 512)
                for k in range(KT):
                    nc.tensor.matmul(
                        slot[:, cs], wqk[p][:, k, 128 * is_k : 128 * is_k + 128],
                        xT[:, k, cs], start=(k == 0), stop=(k == KT - 1),
                    )
                m = KT * is_k + p
                if h2 == 0:
                    t = work.tile([128, S], BF16, tag="t", bufs=2, name=f"t_{nm}")
                    st[(p, is_k, "t")] = t
                else:
                    t = st[(p, is_k, "t")]
                nc.vector.tensor_scalar_add(t[:, cs], slot[:, cs], bqk[:, m : m + 1])

            def rope_chain(p, is_k):
                t = st.pop((p, is_k, "t"))
                ctab = tabs["cosk" if is_k else "cosq"]
                stab = tabs["sink" if is_k else "sinq"]
                sfx = f"{p}_{is_k}"
                t2 = work.tile([128, S], BF16, tag="t2", bufs=2, name=f"t2_{sfx}")
                nc.gpsimd.tensor_mul(t2[:], t[:], t[:])
                st[(p, is_k, "t2")] = t2
                u = work.tile([128, S], BF16, tag="u", bufs=2, name=f"u_{sfx}")
                nc.vector.tensor_mul(u[:], t[:], ctab[:])
                tsh = work.tile([128, S], BF16, tag="tsh", bufs=2, name=f"tsh_{sfx}")
                nc.vector.stream_shuffle(tsh[:], t[:], PAIRSWAP32)
                vv = work.tile([128, S], BF16, tag="vv", bufs=2, name=f"vv_{sfx}")
                nc.vector.tensor_mul(vv[:], tsh[:], stab[:])
                t1 = work.tile([128, S], BF16, tag="t1", bufs=3, name=f"t1_{sfx}")
                nc.gpsimd.tensor_add(t1[:], u[:], vv[:])
                st[(p, is_k, "t1")] = t1

            def sq_mms(p, is_k):
                t2 = st.pop((p, is_k, "t2"))
                sq_ps = st[(p, "sq")]
                for h2 in range(2):
                    nc.tensor.matmul(
                        sq_ps[0:8, :], sqsel[:, 2 * is_k + h2, :],
                        t2[:, 512 * h2 : 512 * h2 + 512],
                        start=(is_k == 0 and h2 == 0), stop=(is_k == 1 and h2 == 1),
                    )

            def sq_alloc(p):
                st[(p, "sq")] = sqp.tile([97, 512], F32, tag="sq", name=f"sq{p}")

            epsc = work.tile([8, 1], F32, tag="epsc")
            nc.gpsimd.memset(epsc[:], EPS)

            def rinv_chain(p):
                sq_ps = st.pop((p, "sq"))
                lnv = work.tile([8, 512], F32, tag="lnv", name=f"lnv{p}")
                nc.scalar.activation(lnv[:], sq_ps[0:8, :], AF.Ln, bias=epsc[:], scale=1.0 / D)
                rinv = work.tile([8, 512], BF16, tag="rinv", bufs=2, name=f"rinv{p}")
                nc.scalar.activation(rinv[:], lnv[:], AF.Exp, scale=-0.5)
                st[(p, "rinv")] = rinv

            def fld_mul(p, is_k):
                rinv = st[(p, "rinv")]
                t1 = st.pop((p, is_k, "t1"))
                nm = f"fld{p}{'k' if is_k else 'q'}"
                slot = ring.tile([128, S], F32, tag="ring", name=nm)
                for h2 in range(2):
                    nc.tensor.matmul(
                        slot[:, 512 * h2 : 512 * h2 + 512],
                        selq[:, 2 * is_k + h2, :], rinv[:],
                        start=True, stop=True,
                    )
                qk = qkp.tile([128, S], BF16, tag="qk", bufs=4, name=f"qk_{nm}")
                nc.vector.tensor_mul(qk[:], t1[:], slot[:])
                st[(p, is_k, "qk")] = qk
                if dbg is not None:
                    nc.sync.dma_start(dbg[f"qk{KT * is_k + p}"][:], qk[:])

            def vproj(j):
                slot = ring.tile([128, S], F32, tag="ring", name=f"vmm{j}")
                for k in range(KT):
                    for lo, hi in ((0, 512), (512, 768)):
                        nc.tensor.matmul(
                            slot[:, lo:hi], xT[:, k, 128 * j : 128 * j + 128],
                            wvT[:, k, lo:hi], start=(k == 0), stop=(k == KT - 1),
                        )
                nc.vector.tensor_add(v_sb[j][:], slot[:, 0:C], vbias[:])
                if dbg is not None:
                    nc.sync.dma_start(dbg[f"v{j}"][:], v_sb[j][:])

            def emit_pv_den(p, j, eA, eB, pv, den):
                hA, hB = 2 * p, 2 * p + 1
                for h2 in range(2):
                    cs = slice(512 * h2, 512 * h2 + 512)
                    nc.tensor.matmul(
                        pv[0:64, cs], v_sb[j][:, 64 * hA : 64 * hA + 64], eA[:, cs],
                        start=(j == 0), stop=(j == ST - 1),
                    )
                    nc.tensor.matmul(
                        pv[64:128, cs], v_sb[j][:, 64 * hB : 64 * hB + 64], eB[:, cs],
                        start=(j == 0), stop=(j == ST - 1),
                    )
                for r, e, cs in (
                    (0, eA, slice(0, 512)), (32, eA, slice(512, 1024)),
                    (64, eB, slice(0, 512)), (96, eB, slice(512, 1024)),
                ):
                    nc.tensor.matmul(
                        den[r : r + 1, :], ones1[:], e[:, cs],
                        start=(j == 0), stop=(j == ST - 1),
                        tile_position=(0, r),
                    )

            def epilogue(p, pv, den):
                # normalize: outT = pv * broadcast(1/den)
                invf = work.tile([97, 512], F32, tag="invf", name=f"invf{p}")
                nc.vector.reciprocal_approx_fast(invf[:], den[:])
                invden = work.tile([97, 512], BF16, tag="invden", name=f"invden{p}")
                nc.gpsimd.tensor_copy(invden[:], invf[:])
                dfld = ring.tile([128, S], F32, tag="ring", name=f"dfld{p}")
                for h2 in range(2):
                    nc.tensor.matmul(
                        dfld[:, 512 * h2 : 512 * h2 + 512], selden[:, h2, :], invden[:],
                        start=True, stop=True,
                    )
                pvc = work.tile([128, S], BF16, tag="pvc", name=f"pvc{p}")
                nc.vector.tensor_copy(pvc[:], pv[:])
                nc.vector.tensor_mul(outT[p][:], pvc[:], dfld[:])
                if dbg is not None:
                    nc.sync.dma_start(dbg[f"ot{p}"][:], outT[p][:])

            def attention(p, tasks, pending_epi=None):
                qt = st.pop((p, 0, "qk"))
                kt = st.pop((p, 1, "qk"))
                pv = pvp.tile([128, S], F32, tag="pv", name=f"pv{p}")
                den = denp.tile([97, 512], F32, tag="den", name=f"den{p}")
                prev = None
                for j in range(ST):
                    # PE queue order per j: PV/den of j-1 and prep tasks go
                    # FIRST so the ring slots for scores(j) free (exp j-1
                    # completion) while the PE is still busy; scores then
                    # issue immediately and the exp stream never starves.
                    if j == 0:
                        # previous pair's normalize runs off the exp critical
                        # path, then den is scrubbed (stale PSUM rows must stay
                        # finite for the 0-weight selden broadcast columns)
                        if pending_epi is not None:
                            pending_epi()
                        nc.vector.memset(den[:], 1.0)
                    if prev is not None:
                        emit_pv_den(p, *prev, pv, den)
                    for fn in tasks.get(j, ()):
                        fn()
                    scA = ring.tile([128, S], F32, tag="ring", name=f"scA{p}_{j}")
                    for h2 in range(2):
                        cs = slice(512 * h2, 512 * h2 + 512)
                        nc.tensor.matmul(
                            scA[:, cs], kt[0:64, 128 * j : 128 * j + 128], qt[0:64, cs],
                            start=True, stop=True,
                        )
                    eA = work.tile([128, S], BF16, tag="exp", bufs=4, name=f"eA{p}_{j}")
                    nc.scalar.activation(eA[:], scA[:], AF.Exp, scale=0.125)
                    scB = ring.tile([128, S], F32, tag="ring", name=f"scB{p}_{j}")
                    for h2 in range(2):
                        cs = slice(512 * h2, 512 * h2 + 512)
                        nc.tensor.matmul(
                            scB[:, cs], kt[64:128, 128 * j : 128 * j + 128], qt[64:128, cs],
                            start=True, stop=True,
                        )
                    eB = work.tile([128, S], BF16, tag="exp", bufs=4, name=f"eB{p}_{j}")
                    nc.scalar.activation(eB[:], scB[:], AF.Exp, scale=0.125)
                    prev = (j, eA, eB)
                emit_pv_den(p, *prev, pv, den)
                return lambda: epilogue(p, pv, den)

            # ---------------- pipeline ----------------
            # pair 0 prep upfront
            sq_alloc(0)
            qkv_half(0, 0, 0)
            qkv_half(0, 0, 1)
            rope_chain(0, 0)
            qkv_half(0, 1, 0)
            qkv_half(0, 1, 1)
            sq_mms(0, 0)
            rope_chain(0, 1)
            sq_mms(0, 1)
            rinv_chain(0)
            fld_mul(0, 0)
            fld_mul(0, 1)
            for j in range(6):
                vproj(j)

            def prep_tasks(p):
                # build pair-(p+1) prep schedule inside attention(p)'s j-loop
                if p + 1 >= NPAIR:
                    return {}
                q = p + 1
                return {
                    1: [lambda: (sq_alloc(q), qkv_half(q, 0, 0))],
                    2: [lambda: (qkv_half(q, 0, 1), rope_chain(q, 0))],
                    3: [lambda: (qkv_half(q, 1, 0), sq_mms(q, 0))],
                    4: [lambda: (qkv_half(q, 1, 1), rope_chain(q, 1))],
                    5: [lambda: (sq_mms(q, 1), rinv_chain(q))],
                    6: [lambda: (fld_mul(q, 0), fld_mul(q, 1))],
                }

            # pair 0's attention streams the last V projections, with pair-1
            # prep on the steady-state schedule shifted two slots later
            t0 = {
                0: [lambda: vproj(6)],
                1: [lambda: vproj(7)],
                3: [lambda: (sq_alloc(1), qkv_half(1, 0, 0))],
                4: [lambda: (qkv_half(1, 0, 1), rope_chain(1, 0))],
                5: [lambda: (qkv_half(1, 1, 0), sq_mms(1, 0))],
                6: [lambda: (qkv_half(1, 1, 1), rope_chain(1, 1))],
                7: [lambda: (sq_mms(1, 1), rinv_chain(1))],
            }
            epi = attention(0, t0)
            fld_mul(1, 0)
            fld_mul(1, 1)
            for p in range(1, NPAIR):
                epi = attention(p, prep_tasks(p), epi)
            epi()

        # ---------------- output projection ----------------
        with tc.tile_pool(name="psy", bufs=2, space="PSUM") as ps_y, \
             tc.tile_pool(name="wy", bufs=2) as wy:
            for mt in range(ST):
                ps = ps_y.tile([128, C], F32, tag="y")
                for k6 in range(KT):
                    nc.tensor.matmul(
                        ps[:, 0:512], outT[k6][:, 128 * mt : 128 * mt + 128],
                        pwT[:, k6, 0:512], start=(k6 == 0), stop=(k6 == KT - 1),
                    )
                    nc.tensor.matmul(
                        ps[:, 512:768], outT[k6][:, 128 * mt : 128 * mt + 128],
                        pwT[:, k6, 512:768], start=(k6 == 0), stop=(k6 == KT - 1),
                    )
                y = wy.tile([128, C], F32, tag="y_sb")
                nc.vector.tensor_add(y[:], ps[:], pbias[:])
                nc.sync.dma_start(out_d[128 * mt : 128 * mt + 128, :], y[:])


LDW_OPT = False  # walrus LDW-opt rejects bass InstLdweights


def _patch_walrus():
    import concourse.bass_utils as _bu
    if getattr(_bu, "_ldwopt_patched", False):
        return
    _orig = _bu.run_command

    def _patched(cmd, **kw):
        if LDW_OPT and isinstance(cmd, list):
            cmd = ["--enable-ldw-opt=true" if c == "--enable-ldw-opt=false" else c for c in cmd]
        return _orig(cmd, **kw)

    _bu.run_command = _patched
    _bu._ldwopt_patched = True


def _patch_act_tables():
    """Restrict Exp/Ln to the combined natural_log_exp set so the table-load
    pass never alternates sets between the softmax exps and the rms-norm
    ln/exp chain (2 reloads per head-pair otherwise). Only the pass's choice
    is narrowed; set ids/contents still match act_info.json."""
    import functools
    import concourse.hw_specs as _hs
    import concourse.bacc as _bacc
    if getattr(_hs, "_act_tbl_patched", False):
        return
    _orig = _hs.get_activation_tables

    @functools.cache
    def _patched(arch):
        t = {k: set(v) for k, v in _orig(arch).items()}
        for name, funcs in t.items():
            if name != "natural_log_exp_and_others":
                funcs.discard(AF.Exp)
                funcs.discard(AF.Ln)
        return t

    _hs.get_activation_tables = _patched
    _bacc.get_activation_tables = _patched
    _hs._act_tbl_patched = True


def build():
    if "nc" in _CACHE:
        return _CACHE["nc"]
    _patch_walrus()
    _patch_act_tables()
    nc = bacc.Bacc("TRN2", target_bir_lowering=False, debug=False)
    d = {}

    def din(name, shape, dt):
        d[name] = nc.dram_tensor(name, shape, dt, kind="ExternalInput").ap()

    din("xT", [C, S], BF16)
    din("wqk", [NPAIR * C, 256], BF16)
    din("wvT", [C, C], BF16)
    din("pwT", [C, C], BF16)
    din("bqk", [128, 2 * KT], F32)
    din("vbias", [128, C], F32)
    din("pbias", [128, C], F32)
    din("cosq", [128, S], BF16)
    din("sinq", [128, S], BF16)
    din("cosk", [128, S], BF16)
    din("sink", [128, S], BF16)
    din("ones1", [128, 1], BF16)
    din("sqsel", [128, 32], BF16)
    din("selq", [8, 512], BF16)
    din("selden", [97, 256], BF16)
    out_d = nc.dram_tensor("out", [S, C], F32, kind="ExternalOutput").ap()
    dbg = None
    if DEBUG:
        dbg = {}
        for m in range(2 * KT):
            dbg[f"qk{m}"] = nc.dram_tensor(f"dbg_qk{m}", [128, S], BF16, kind="ExternalOutput").ap()
        for j in range(ST):
            dbg[f"v{j}"] = nc.dram_tensor(f"dbg_v{j}", [128, C], BF16, kind="ExternalOutput").ap()
        for p in range(NPAIR):
            dbg[f"ot{p}"] = nc.dram_tensor(f"dbg_ot{p}", [128, S], BF16, kind="ExternalOutput").ap()

    with tile.TileContext(nc) as tc:
        _graph(tc, d, out_d, dbg)
    nc.compile()
    _CACHE["nc"] = nc
    return nc


def make_in_maps(x, qkv_w, qkv_b, q_norm_w, k_norm_w, proj_w, proj_b):
    shared = _prep_shared(qkv_w, qkv_b, q_norm_w, k_norm_w, proj_w, proj_b)
    x = np.asarray(x, np.float32)
    in_maps = []
    for b in range(NCORES):
        m = dict(shared)
        m["xT"] = np.ascontiguousarray(x[b].T).astype(NP_BF16)
        in_maps.append(m)
    return in_maps


def run(in_maps, trace=False, **kw):
    nc = build()
    return run_bass_kernel_spmd(nc, in_maps, core_ids=list(range(NCORES)), trace=trace, **kw)


def kernel(x, qkv_w, qkv_b, q_norm_w, k_norm_w, proj_w, proj_b):
    in_maps = make_in_maps(x, qkv_w, qkv_b, q_norm_w, k_norm_w, proj_w, proj_b)
    res = run(in_maps)
    return np.stack([np.asarray(res.results[i]["out"]) for i in range(NCORES)]).astype(np.float32)


if __name__ == "__main__":
    rng = np.random.default_rng(0)
    ins = {
        "x": rng.standard_normal((B, S, C)).astype(np.float32),
        "qkv_w": (rng.standard_normal((3 * C, C)) * C**-0.5).astype(np.float32),
        "qkv_b": (rng.standard_normal(3 * C) * 0.02).astype(np.float32),
        "q_norm_w": np.ones(D, np.float32),
        "k_norm_w": np.ones(D, np.float32),
        "proj_w": (rng.standard_normal((C, C)) * C**-0.5).astype(np.float32),
        "proj_b": (rng.standard_normal(C) * 0.02).astype(np.float32),
    }
    y = kernel(**ins)
    print("out", y.shape, y.dtype)


# revision 20
# speedup vs baseline: 1.4883x; 1.4883x over previous
"""Trainium2 Bass kernel for nn_Attention_26628797235884.

12-head attention block (qkv proj + per-head RMS norm + 2D RoPE + softmax
attention + output proj), batch 8 x seq 1024 x dim 768, data-parallel over
batch across 8 NeuronCores (batch b -> core b, weights replicated).

v2: software-pipelined over the 6 head-pairs so the TensorEngine work of
pair p+1 (QKV matmuls, rms-norm group sums, rope) fills the PE idle time
while the ScalarEngine runs the softmax exps of pair p (ACT is the serial
bottleneck of attention at ~19us/pair). Other key points:
  - scores computed transposed [sk, sq] in two K=64 row-tiled matmuls that
    run concurrently in the PE array (head A rows 0:64, head B rows 64:128).
  - PV col-tiled: head A -> psum partitions 0:64, head B -> 64:128,
    concurrently (M=64 each); softmax denominators from four M=1 matmuls
    (ones lhsT) col-tiled at psum partitions 0/32/64/96 of one bank.
  - rms rsqrt via exp(-0.5*ln(x)) so the ACT engine never leaves the
    ln+exp activation-table set (sqrt would force a table reload per pair).
  - all PSUM traffic fits 8 banks: 2-slot [128,1024] ring (scores, qkv
    groups, rope-norm fields, denom broadcast) + pv(2) + den(1) + sq(1).
All matmuls bf16 (inputs pre-cast on host), fp32 accumulation.
"""

import sys

import numpy as np
import ml_dtypes

try:
    import concourse.bass as bass  # noqa: F401
except ImportError:  # pragma: no cover
    sys.path.insert(0, "/opt/trn_rl_repo")

import concourse.tile as tile
from concourse import bacc, mybir
from concourse.bass_utils import run_bass_kernel_spmd

BF16 = mybir.dt.bfloat16
F32 = mybir.dt.float32
AF = mybir.ActivationFunctionType
NP_BF16 = ml_dtypes.bfloat16

B, S, C, H, D = 8, 1024, 768, 12, 64
KT = C // 128           # 6 contraction tiles over the model dim
ST = S // 128           # 8 seq tiles
NPAIR = 6               # head pairs
NCORES = 8
EPS = 1e-6
PAIRSWAP32 = [i ^ 1 for i in range(32)]

_CACHE = {}
DEBUG = False


# --------------------------------------------------------------------------
# host-side constant prep
# --------------------------------------------------------------------------

def _rope_tables():
    ROPE_DIM, PT_SEQ, FT_SEQ, THETA = 32, 16, 32, 10000.0
    freqs = 1.0 / (THETA ** (np.arange(0, ROPE_DIM, 2, dtype=np.float32)[: ROPE_DIM // 2] / ROPE_DIM))
    t = np.arange(FT_SEQ, dtype=np.float32) / FT_SEQ * PT_SEQ
    f = np.einsum("i,j->ij", t, freqs)
    f = np.repeat(f, 2, axis=-1)
    fh = np.broadcast_to(f[:, None, :], (FT_SEQ, FT_SEQ, ROPE_DIM))
    fw = np.broadcast_to(f[None, :, :], (FT_SEQ, FT_SEQ, ROPE_DIM))
    f2 = np.concatenate([fh, fw], axis=-1).reshape(FT_SEQ * FT_SEQ, 2 * ROPE_DIM)
    return np.cos(f2).astype(np.float32), np.sin(f2).astype(np.float32)


def _prep_shared(qkv_w, qkv_b, q_norm_w, k_norm_w, proj_w, proj_b):
    f32 = np.float32
    cos, sin = _rope_tables()                 # [S, D]
    pair = np.arange(D) ^ 1
    sa = sin.copy()
    sa[:, 0::2] *= -1.0                       # sign-folded sin for rotate_half

    def mk(tab, w):                           # -> [128, S] bf16, 2 heads stacked
        t = (tab * w[None, :]).T.astype(f32)  # [64, S]
        return np.ascontiguousarray(np.vstack([t, t])).astype(NP_BF16)

    qw = np.asarray(q_norm_w, f32)
    kw = np.asarray(k_norm_w, f32)
    wqkT = np.asarray(qkv_w, f32)[: 2 * C].T  # [C, 2C]; cols: q-tiles then k-tiles
    wqk = np.concatenate(
        [np.concatenate([wqkT[:, 128 * p : 128 * p + 128],
                         wqkT[:, C + 128 * p : C + 128 * p + 128]], axis=1)[None]
         for p in range(NPAIR)], axis=0)      # [6, C, 256] pair-major
    shared = {
        "wqk": np.ascontiguousarray(wqk.reshape(NPAIR * C, 256)).astype(NP_BF16),
        "wvT": np.ascontiguousarray(np.asarray(qkv_w, f32)[2 * C :].T).astype(NP_BF16),
        "pwT": np.ascontiguousarray(np.asarray(proj_w, f32).T).astype(NP_BF16),
        "cosq": mk(cos, qw),
        "sinq": mk(sa, qw[pair]),
        "cosk": mk(cos, kw),
        "sink": mk(sa, kw[pair]),
    }
    b = np.asarray(qkv_b, f32)
    shared["bqk"] = np.ascontiguousarray(b[: 2 * C].reshape(2 * KT, 128).T)  # [128, 12]
    shared["vbias"] = np.ascontiguousarray(np.tile(b[2 * C :][None, :], (128, 1)))
    shared["pbias"] = np.ascontiguousarray(np.tile(np.asarray(proj_b, f32)[None, :], (128, 1)))
    shared["ones1"] = np.ones((128, 1), NP_BF16)
    # sqsel[:, 2*is_k + h2, :] is a [128, 8] indicator lhsT: partition d of an
    # m-tile's t^2 chunk (h2) accumulates into ssq row 4*is_k + 2*(d//64) + h2.
    dd = np.arange(128)
    sqsel = np.zeros((128, 4, 8), NP_BF16)
    for is_k in range(2):
        for h2 in range(2):
            sqsel[dd, 2 * is_k + h2, 4 * is_k + 2 * (dd // 64) + h2] = 1
    shared["sqsel"] = np.ascontiguousarray(sqsel.reshape(128, 32))
    # selq[:, 2*is_k + h2, :] is an [8, 128] lhsT broadcasting rinv row
    # 4*is_k + 2*(P//64) + h2 to field partition P (chunk h2).
    P = np.arange(128)
    selq = np.zeros((8, 4, 128), NP_BF16)
    for is_k in range(2):
        for h2 in range(2):
            selq[4 * is_k + 2 * (P // 64) + h2, 2 * is_k + h2, P] = 1
    shared["selq"] = np.ascontiguousarray(selq.reshape(8, 512))
    # selden[:, h2, :]: [97, 128] lhsT broadcasting inv-denominator psum row
    # (0 if P<64 else 64) + 32*h2 to field partition P (chunk h2).
    selden = np.zeros((97, 2, 128), NP_BF16)
    for h2 in range(2):
        selden[(P // 64) * 64 + 32 * h2, h2, P] = 1
    shared["selden"] = np.ascontiguousarray(selden.reshape(97, 256))
    return shared


# --------------------------------------------------------------------------
# device graph
# --------------------------------------------------------------------------

def _graph(tc, d, out_d, dbg=None):
    nc = tc.nc
    from contextlib import ExitStack

    with ExitStack() as big:
        main = big.enter_context(tc.tile_pool(name="main", bufs=1))
        work = big.enter_context(tc.tile_pool(name="work", bufs=1))

        # ---------------- persistent SBUF ----------------
        xT = main.tile([128, KT, S], BF16, tag="xT")
        wqk = [main.tile([128, KT, 256], BF16, tag=f"wqk{p}", name=f"wqk{p}") for p in range(NPAIR)]
        wvT = main.tile([128, KT, C], BF16, tag="wvT")
        pwT = main.tile([128, KT, C], BF16, tag="pwT")
        tabs = {}
        for nm in ("cosq", "sinq", "cosk", "sink"):
            tabs[nm] = main.tile([128, S], BF16, tag=nm, name=nm)
        bqk = main.tile([128, 2 * KT], F32, tag="bqk")
        vbias = main.tile([128, C], F32, tag="vbias")
        pbias = main.tile([128, C], F32, tag="pbias")
        ones1 = main.tile([128, 1], BF16, tag="ones1")
        sqsel = main.tile([128, 4, 8], BF16, tag="sqsel")
        selq = main.tile([8, 4, 128], BF16, tag="selq")
        selden = main.tile([97, 2, 128], BF16, tag="selden")
        v_sb = [main.tile([128, C], BF16, tag=f"v{j}", name=f"v{j}") for j in range(ST)]
        outT = [main.tile([128, S], BF16, tag=f"ot{p}", name=f"ot{p}") for p in range(NPAIR)]

        # ---------------- DMAs, priority order ----------------
        xT_r = d["xT"].rearrange("(k p) s -> p k s", p=128)
        for k in range(KT):
            nc.sync.dma_start(xT[:, k], xT_r[:, k])
        wqk_r = d["wqk"].rearrange("(pp kk p) c -> pp p kk c", pp=NPAIR, p=128)
        nc.sync.dma_start(wqk[0][:], wqk_r[0])
        nc.sync.dma_start(bqk[:], d["bqk"][:])
        for nm in tabs:
            nc.sync.dma_start(tabs[nm][:], d[nm][:])
        nc.sync.dma_start(sqsel[:], d["sqsel"].rearrange("p (b e) -> p b e", b=4))
        nc.sync.dma_start(selq[:], d["selq"].rearrange("p (b e) -> p b e", b=4))
        nc.sync.dma_start(ones1[:], d["ones1"][:])
        wvT_r = d["wvT"].rearrange("(k p) o -> p k o", p=128)
        for k in range(KT):
            nc.sync.dma_start(wvT[:, k], wvT_r[:, k])
        nc.sync.dma_start(vbias[:], d["vbias"][:])
        nc.sync.dma_start(wqk[1][:], wqk_r[1])
        nc.sync.dma_start(selden[:], d["selden"].rearrange("p (b e) -> p b e", b=2))
        for p in range(2, NPAIR):
            nc.sync.dma_start(wqk[p][:], wqk_r[p])
        pwT_r = d["pwT"].rearrange("(k p) o -> p k o", p=128)
        nc.sync.dma_start(pwT[:], pwT_r[:])
        nc.sync.dma_start(pbias[:], d["pbias"][:])

        st = {}  # per-pair pipeline state: t1/qk tiles etc.

        with ExitStack() as att:
            ring = att.enter_context(tc.tile_pool(name="ring", bufs=2, space="PSUM"))
            pvp = att.enter_context(tc.tile_pool(name="pvp", bufs=1, space="PSUM"))
            denp = att.enter_context(tc.tile_pool(name="denp", bufs=1, space="PSUM"))
            sqp = att.enter_context(tc.tile_pool(name="sqp", bufs=1, space="PSUM"))
            qkp = att.enter_context(tc.tile_pool(name="qkp", bufs=1))

            # ---------------- emission helpers ----------------
            def qkv_quarter(p, is_k, c):
                # quarter-width QKV accumulation in its own ring slot so the
                # PE cost per attention j-slot stays under the exp period
                nm = f"qkv{p}{'k' if is_k else 'q'}{c}"
                slot = ring.tile([128, S], F32, tag="ring", name=nm)
                cs = slice(256 * c, 256 * c + 256)
                for k in range(KT):
                    nc.tensor.matmul(
                        slot[:, cs], wqk[p][:, k, 128 * is_k : 128 * is_k + 128],
                        xT[:, k, cs], start=(k == 0), stop=(k == KT - 1),
                    )
                m = KT * is_k + p
                if c == 0:
                    t = work.tile([128, S], BF16, tag="t", bufs=2, name=f"t_{nm}")
                    st[(p, is_k, "t")] = t
                else:
                    t = st[(p, is_k, "t")]
                nc.vector.tensor_scalar_add(t[:, cs], slot[:, cs], bqk[:, m : m + 1])

            def rope_chain(p, is_k):
                t = st.pop((p, is_k, "t"))
                ctab = tabs["cosk" if is_k else "cosq"]
                stab = tabs["sink" if is_k else "sinq"]
                sfx = f"{p}_{is_k}"
                t2 = work.tile([128, S], BF16, tag="t2", bufs=2, name=f"t2_{sfx}")
                nc.gpsimd.tensor_mul(t2[:], t[:], t[:])
                st[(p, is_k, "t2")] = t2
                u = work.tile([128, S], BF16, tag="u", bufs=2, name=f"u_{sfx}")
                nc.vector.tensor_mul(u[:], t[:], ctab[:])
                tsh = work.tile([128, S], BF16, tag="tsh", bufs=2, name=f"tsh_{sfx}")
                nc.vector.stream_shuffle(tsh[:], t[:], PAIRSWAP32)
                vv = work.tile([128, S], BF16, tag="vv", bufs=2, name=f"vv_{sfx}")
                nc.vector.tensor_mul(vv[:], tsh[:], stab[:])
                t1 = work.tile([128, S], BF16, tag="t1", bufs=3, name=f"t1_{sfx}")
                nc.gpsimd.tensor_add(t1[:], u[:], vv[:])
                st[(p, is_k, "t1")] = t1

            def sq_mms(p, is_k):
                t2 = st.pop((p, is_k, "t2"))
                sq_ps = st[(p, "sq")]
                for h2 in range(2):
                    nc.tensor.matmul(
                        sq_ps[0:8, :], sqsel[:, 2 * is_k + h2, :],
                        t2[:, 512 * h2 : 512 * h2 + 512],
                        start=(is_k == 0 and h2 == 0), stop=(is_k == 1 and h2 == 1),
                    )

            def sq_alloc(p):
                st[(p, "sq")] = sqp.tile([97, 512], F32, tag="sq", name=f"sq{p}")

            epsc = work.tile([8, 1], F32, tag="epsc")
            nc.gpsimd.memset(epsc[:], EPS)

            def rinv_chain(p):
                sq_ps = st.pop((p, "sq"))
                lnv = work.tile([8, 512], F32, tag="lnv", name=f"lnv{p}")
                nc.scalar.activation(lnv[:], sq_ps[0:8, :], AF.Ln, bias=epsc[:], scale=1.0 / D)
                rinv = work.tile([8, 512], BF16, tag="rinv", bufs=2, name=f"rinv{p}")
                nc.scalar.activation(rinv[:], lnv[:], AF.Exp, scale=-0.5)
                st[(p, "rinv")] = rinv

            def fld_mul(p, is_k):
                rinv = st[(p, "rinv")]
                t1 = st.pop((p, is_k, "t1"))
                nm = f"fld{p}{'k' if is_k else 'q'}"
                slot = ring.tile([128, S], F32, tag="ring", name=nm)
                for h2 in range(2):
                    nc.tensor.matmul(
                        slot[:, 512 * h2 : 512 * h2 + 512],
                        selq[:, 2 * is_k + h2, :], rinv[:],
                        start=True, stop=True,
                    )
                qk = qkp.tile([128, S], BF16, tag="qk", bufs=4, name=f"qk_{nm}")
                nc.vector.tensor_mul(qk[:], t1[:], slot[:])
                st[(p, is_k, "qk")] = qk
                if dbg is not None:
                    nc.sync.dma_start(dbg[f"qk{KT * is_k + p}"][:], qk[:])

            def vproj(j):
                slot = ring.tile([128, S], F32, tag="ring", name=f"vmm{j}")
                for k in range(KT):
                    for lo, hi in ((0, 512), (512, 768)):
                        nc.tensor.matmul(
                            slot[:, lo:hi], xT[:, k, 128 * j : 128 * j + 128],
                            wvT[:, k, lo:hi], start=(k == 0), stop=(k == KT - 1),
                        )
                nc.vector.tensor_add(v_sb[j][:], slot[:, 0:C], vbias[:])
                if dbg is not None:
                    nc.sync.dma_start(dbg[f"v{j}"][:], v_sb[j][:])

            def emit_pv_den(p, is_b, j, e, pv, den):
                h = 2 * p + is_b
                po = 64 * is_b
                for h2 in range(2):
                    cs = slice(512 * h2, 512 * h2 + 512)
                    nc.tensor.matmul(
                        pv[po : po + 64, cs], v_sb[j][:, 64 * h : 64 * h + 64], e[:, cs],
                        start=(j == 0), stop=(j == ST - 1),
                    )
                for r, cs in ((64 * is_b, slice(0, 512)), (64 * is_b + 32, slice(512, 1024))):
                    nc.tensor.matmul(
                        den[r : r + 1, :], ones1[:], e[:, cs],
                        start=(j == 0), stop=(j == ST - 1),
                        tile_position=(0, r),
                    )

            def epilogue(p, pv, den):
                # normalize: outT = pv * broadcast(1/den)
                invf = work.tile([97, 512], F32, tag="invf", name=f"invf{p}")
                nc.vector.reciprocal_approx_fast(invf[:], den[:])
                invden = work.tile([97, 512], BF16, tag="invden", name=f"invden{p}")
                nc.vector.tensor_copy(invden[:], invf[:])
                dfld = ring.tile([128, S], F32, tag="ring", name=f"dfld{p}")
                for h2 in range(2):
                    nc.tensor.matmul(
                        dfld[:, 512 * h2 : 512 * h2 + 512], selden[:, h2, :], invden[:],
                        start=True, stop=True,
                    )
                pvc = work.tile([128, S], BF16, tag="pvc", name=f"pvc{p}")
                nc.vector.tensor_copy(pvc[:], pv[:])
                nc.vector.tensor_mul(outT[p][:], pvc[:], dfld[:])
                if dbg is not None:
                    nc.sync.dma_start(dbg[f"ot{p}"][:], outT[p][:])

            def attention(p, tasks, pending_epi=None):
                qt = st.pop((p, 0, "qk"))
                kt = st.pop((p, 1, "qk"))
                pv = pvp.tile([128, S], F32, tag="pv", name=f"pv{p}")
                den = denp.tile([97, 512], F32, tag="den", name=f"den{p}")
                prev = None
                for j in range(ST):
                    # per-stream PE order: scores(j) for a stream are gated on
                    # the same event (exp j-1 of that stream) as its PV/den of
                    # j-1, so each gate releases scores first (keeping ACT
                    # fed), then the already-unblocked PV/den; ungated prep
                    # tasks come last and soak up any PE wait.
                    scA = ring.tile([128, S], F32, tag="ring", name=f"scA{p}_{j}")
                    for h2 in range(2):
                        cs = slice(512 * h2, 512 * h2 + 512)
                        nc.tensor.matmul(
                            scA[:, cs], kt[0:64, 128 * j : 128 * j + 128], qt[0:64, cs],
                            start=True, stop=True,
                        )
                    eA = work.tile([128, S], BF16, tag="exp", bufs=4, name=f"eA{p}_{j}")
                    nc.scalar.activation(eA[:], scA[:], AF.Exp, scale=0.125)
                    if prev is not None:
                        emit_pv_den(p, 0, prev[0], prev[1], pv, den)
                    scB = ring.tile([128, S], F32, tag="ring", name=f"scB{p}_{j}")
                    for h2 in range(2):
                        cs = slice(512 * h2, 512 * h2 + 512)
                        nc.tensor.matmul(
                            scB[:, cs], kt[64:128, 128 * j : 128 * j + 128], qt[64:128, cs],
                            start=True, stop=True,
                        )
                    eB = work.tile([128, S], BF16, tag="exp", bufs=4, name=f"eB{p}_{j}")
                    nc.scalar.activation(eB[:], scB[:], AF.Exp, scale=0.125)
                    if prev is not None:
                        emit_pv_den(p, 1, prev[0], prev[2], pv, den)
                    if j == 0:
                        # previous pair's normalize runs off the exp critical
                        # path, then den is scrubbed (stale PSUM rows must stay
                        # finite for the 0-weight selden broadcast columns)
                        if pending_epi is not None:
                            pending_epi()
                        nc.vector.memset(den[:], 1.0)
                    for fn in tasks.get(j, ()):
                        fn()
                    prev = (j, eA, eB)
                emit_pv_den(p, 0, prev[0], prev[1], pv, den)
                emit_pv_den(p, 1, prev[0], prev[2], pv, den)
                return lambda: epilogue(p, pv, den)

            # ---------------- pipeline ----------------
            # pair 0 prep upfront
            sq_alloc(0)
            for c in range(4):
                qkv_quarter(0, 0, c)
            rope_chain(0, 0)
            for c in range(4):
                qkv_quarter(0, 1, c)
            sq_mms(0, 0)
            rope_chain(0, 1)
            sq_mms(0, 1)
            rinv_chain(0)
            fld_mul(0, 0)
            fld_mul(0, 1)
            vproj(0)
            vproj(1)

            def prep_tasks(p):
                # build pair-(p+1) prep schedule inside attention(p)'s j-loop:
                # q-tile quarters j0..j3, k-tile quarters j3..j5 (doubled),
                # so the k-side norm/rope chain finishes before att(p+1)
                if p + 1 >= NPAIR:
                    return {}
                q = p + 1
                return {
                    0: [lambda: (sq_alloc(q), qkv_quarter(q, 0, 0))],
                    1: [lambda: qkv_quarter(q, 0, 1)],
                    2: [lambda: qkv_quarter(q, 0, 2)],
                    3: [lambda: (qkv_quarter(q, 0, 3), rope_chain(q, 0),
                                 qkv_quarter(q, 1, 0))],
                    4: [lambda: (qkv_quarter(q, 1, 1), qkv_quarter(q, 1, 2),
                                 sq_mms(q, 0))],
                    5: [lambda: (qkv_quarter(q, 1, 3), rope_chain(q, 1))],
                    6: [lambda: (sq_mms(q, 1), rinv_chain(q))],
                    7: [lambda: (fld_mul(q, 0), fld_mul(q, 1))],
                }

            # pair 0's attention also streams the remaining V projections;
            # pair-1 prep is compressed into the later slots
            t0 = {
                0: [lambda: vproj(2)],
                1: [lambda: vproj(3)],
                2: [lambda: (vproj(4), sq_alloc(1), qkv_quarter(1, 0, 0))],
                3: [lambda: (vproj(5), qkv_quarter(1, 0, 1))],
                4: [lambda: (vproj(6), qkv_quarter(1, 0, 2))],
                5: [lambda: (vproj(7), qkv_quarter(1, 0, 3), rope_chain(1, 0),
                             qkv_quarter(1, 1, 0))],
                6: [lambda: (qkv_quarter(1, 1, 1), qkv_quarter(1, 1, 2),
                             sq_mms(1, 0))],
                7: [lambda: (qkv_quarter(1, 1, 3), rope_chain(1, 1),
                             sq_mms(1, 1), rinv_chain(1))],
            }
            epi = attention(0, t0)
            fld_mul(1, 0)
            fld_mul(1, 1)
            for p in range(1, NPAIR):
                epi = attention(p, prep_tasks(p), epi)
            epi()

        # ---------------- output projection ----------------
        with tc.tile_pool(name="psy", bufs=2, space="PSUM") as ps_y, \
             tc.tile_pool(name="wy", bufs=2) as wy:
            for mt in range(ST):
                ps = ps_y.tile([128, C], F32, tag="y")
                for k6 in range(KT):
                    nc.tensor.matmul(
                        ps[:, 0:512], outT[k6][:, 128 * mt : 128 * mt + 128],
                        pwT[:, k6, 0:512], start=(k6 == 0), stop=(k6 == KT - 1),
                    )
                    nc.tensor.matmul(
                        ps[:, 512:768], outT[k6][:, 128 * mt : 128 * mt + 128],
                        pwT[:, k6, 512:768], start=(k6 == 0), stop=(k6 == KT - 1),
                    )
                y = wy.tile([128, C], F32, tag="y_sb")
                nc.vector.tensor_add(y[:], ps[:], pbias[:])
                nc.sync.dma_start(out_d[128 * mt : 128 * mt + 128, :], y[:])


LDW_OPT = False  # walrus LDW-opt rejects bass InstLdweights


def _patch_walrus():
    import concourse.bass_utils as _bu
    if getattr(_bu, "_ldwopt_patched", False):
        return
    _orig = _bu.run_command

    def _patched(cmd, **kw):
        if LDW_OPT and isinstance(cmd, list):
            cmd = ["--enable-ldw-opt=true" if c == "--enable-ldw-opt=false" else c for c in cmd]
        return _orig(cmd, **kw)

    _bu.run_command = _patched
    _bu._ldwopt_patched = True


def _patch_act_tables():
    """Restrict Exp/Ln to the combined natural_log_exp set so the table-load
    pass never alternates sets between the softmax exps and the rms-norm
    ln/exp chain (2 reloads per head-pair otherwise). Only the pass's choice
    is narrowed; set ids/contents still match act_info.json."""
    import functools
    import concourse.hw_specs as _hs
    import concourse.bacc as _bacc
    if getattr(_hs, "_act_tbl_patched", False):
        return
    _orig = _hs.get_activation_tables

    @functools.cache
    def _patched(arch):
        t = {k: set(v) for k, v in _orig(arch).items()}
        for name, funcs in t.items():
            if name != "natural_log_exp_and_others":
                funcs.discard(AF.Exp)
                funcs.discard(AF.Ln)
        return t

    _hs.get_activation_tables = _patched
    _bacc.get_activation_tables = _patched
    _hs._act_tbl_patched = True


def build():
    if "nc" in _CACHE:
        return _CACHE["nc"]
    _patch_walrus()
    _patch_act_tables()
    nc = bacc.Bacc("TRN2", target_bir_lowering=False, debug=False)
    d = {}

    def din(name, shape, dt):
        d[name] = nc.dram_tensor(name, shape, dt, kind="ExternalInput").ap()

    din("xT", [C, S], BF16)
    din("wqk", [NPAIR * C, 256], BF16)
    din("wvT", [C, C], BF16)
    din("pwT", [C, C], BF16)
    din("bqk", [128, 2 * KT], F32)
    din("vbias", [128, C], F32)
    din("pbias", [128, C], F32)
    din("cosq", [128, S], BF16)
    din("sinq", [128, S], BF16)
    din("cosk", [128, S], BF16)
    din("sink", [128, S], BF16)
    din("ones1", [128, 1], BF16)
    din("sqsel", [128, 32], BF16)
    din("selq", [8, 512], BF16)
    din("selden", [97, 256], BF16)
    out_d = nc.dram_tensor("out", [S, C], F32, kind="ExternalOutput").ap()
    dbg = None
    if DEBUG:
        dbg = {}
        for m in range(2 * KT):
            dbg[f"qk{m}"] = nc.dram_tensor(f"dbg_qk{m}", [128, S], BF16, kind="ExternalOutput").ap()
        for j in range(ST):
            dbg[f"v{j}"] = nc.dram_tensor(f"dbg_v{j}", [128, C], BF16, kind="ExternalOutput").ap()
        for p in range(NPAIR):
            dbg[f"ot{p}"] = nc.dram_tensor(f"dbg_ot{p}", [128, S], BF16, kind="ExternalOutput").ap()

    with tile.TileContext(nc) as tc:
        _graph(tc, d, out_d, dbg)
    nc.compile()
    _CACHE["nc"] = nc
    return nc


def make_in_maps(x, qkv_w, qkv_b, q_norm_w, k_norm_w, proj_w, proj_b):
    shared = _prep_shared(qkv_w, qkv_b, q_norm_w, k_norm_w, proj_w, proj_b)
    x = np.asarray(x, np.float32)
    in_maps = []
    for b in range(NCORES):
        m = dict(shared)
        m["xT"] = np.ascontiguousarray(x[b].T).astype(NP_BF16)
        in_maps.append(m)
    return in_maps


def run(in_maps, trace=False, **kw):
    nc = build()
    return run_bass_kernel_spmd(nc, in_maps, core_ids=list(range(NCORES)), trace=trace, **kw)


def kernel(x, qkv_w, qkv_b, q_norm_w, k_norm_w, proj_w, proj_b):
    in_maps = make_in_maps(x, qkv_w, qkv_b, q_norm_w, k_norm_w, proj_w, proj_b)
    res = run(in_maps)
    return np.stack([np.asarray(res.results[i]["out"]) for i in range(NCORES)]).astype(np.float32)


if __name__ == "__main__":
    rng = np.random.default_rng(0)
    ins = {
        "x": rng.standard_normal((B, S, C)).astype(np.float32),
        "qkv_w": (rng.standard_normal((3 * C, C)) * C**-0.5).astype(np.float32),
        "qkv_b": (rng.standard_normal(3 * C) * 0.02).astype(np.float32),
        "q_norm_w": np.ones(D, np.float32),
        "k_norm_w": np.ones(D, np.float32),
        "proj_w": (rng.standard_normal((C, C)) * C**-0.5).astype(np.float32),
        "proj_b": (rng.standard_normal(C) * 0.02).astype(np.float32),
    }
    y = kernel(**ins)
    print("out", y.shape, y.dtype)
